# revision 17
# baseline (speedup 1.0000x reference)
"""Cross-attention Trainium2 kernel (8 NeuronCores, data-parallel).

Problem: B=4, C=64, H=64, W=64.
  q = conv1x1(v1, wq, bq); k = conv1x1(v2, wk, bk); v = conv1x1(v2, wv, bv)
  tokens n = (c, h) pairs (N = C*H = 4096), feature dim = W = 64
  out = softmax(q @ k^T) @ v

Sharding: core i handles batch b = i//2 and the q-token half h in
[32*(i%2), 32*(i%2+1)).  Every core needs the full v2[b] (k/v side) but only
its h-slice of v1[b] (q side).  No collectives.

Per-core algorithm:
  - scores computed TRANSPOSED: sT[j, i] = k_j . q_i with k-tokens j on
    partitions; after exp the tile is exactly the stationary-operand layout
    the P@V matmul needs (no attention-matrix transpose ever).
  - no max subtraction (|s| <= ~74 here; exp fits fp32); softmax denominator
    via a ones-column appended to V.
  - scores contraction is W=64 (half the PE array), so TWO k-token blocks
    are packed into the array concurrently via tile_position row groups:
    kT2/qT2 hold duplicated/feature-major data on partitions 0-63 and
    64-127.  This makes the f32r scores matmuls SBUF-bandwidth-bound and
    insensitive to the HAM clock state.
  - f32r for projections + scores, bf16 for exp weights and V, fp32 PSUM.
"""

import numpy as np

B, C, H, W = 4, 64, 64, 64
HH = H // 2            # h-rows per core (q-token half)
NQ = C * HH            # q tokens per core = 2048
NK = C * H             # k tokens = 4096
JB = NK // 128         # 32 j-blocks of 128 k-tokens
NP = JB // 2           # 16 row-packed j-block pairs
IP = 512               # i-span per pass (4 passes)
NCORES = 8

_CACHE = {}


def _build_nc():
    from contextlib import ExitStack

    import concourse.bass as bass
    import concourse.tile as tile
    from concourse import bacc, mybir
    from concourse.bass import ts
    from concourse.masks import make_identity

    F32 = mybir.dt.float32
    F32R = mybir.dt.float32r
    BF16 = mybir.dt.bfloat16
    AF = mybir.ActivationFunctionType

    nc = bacc.Bacc(trn_type="TRN2", target_bir_lowering=False)

    x1 = nc.declare_dram_parameter("x1", [C, HH * W], F32, False)
    x2 = nc.declare_dram_parameter("x2", [C, H * W], F32, False)
    wq_d = nc.declare_dram_parameter("wq", [C, C], F32, False)
    wk_d = nc.declare_dram_parameter("wk", [C, C], F32, False)
    wv_d = nc.declare_dram_parameter("wv", [C, C], F32, False)
    bq_d = nc.declare_dram_parameter("bq", [1, C], F32, False)
    bk_d = nc.declare_dram_parameter("bk", [1, C], F32, False)
    bv_d = nc.declare_dram_parameter("bv", [1, C], F32, False)
    out_d = nc.declare_dram_parameter("out", [C, HH, W], F32, True)

    with ExitStack() as ctx:
        tc = ctx.enter_context(tile.TileContext(nc))
        cp = ctx.enter_context(tc.tile_pool(name="const", bufs=1))

        ident = cp.tile([128, 128], F32)
        make_identity(nc, ident[:, :])

        # prewarm the exp table set while input DMAs run
        warm = cp.tile([128, 2], F32)
        nc.vector.memset(warm[:, :], 0.0)
        nc.scalar.activation(warm[:, 0:1], warm[:, 1:2], AF.Exp)

        # f32r matmul operands must be engine-rounded; DMA can't round, so
        # DMA to fp32 staging then copy (chunked to bound per-inst waits).
        x1_st = cp.tile([C + 1, HH * W], F32)
        x2_st = cp.tile([C + 1, H * W], F32)
        x1_sb = cp.tile([C + 1, HH * W], F32R)
        x2_sb = cp.tile([C + 1, H * W], F32R)
        nc.vector.memset(x1_st[C : C + 1, :], 1.0)   # ones row -> bias via matmul
        nc.vector.memset(x2_st[C : C + 1, :], 1.0)
        nc.sync.dma_start(x1_st[0:C, :], x1[:, :])
        nc.sync.dma_start(x2_st[0:C, :], x2[:, :])
        for c in range(HH * W // 1024):
            nc.vector.tensor_copy(x1_sb[:, ts(c, 1024)], x1_st[:, ts(c, 1024)])
        for c in range(H * W // 1024):
            nc.vector.tensor_copy(x2_sb[:, ts(c, 1024)], x2_st[:, ts(c, 1024)])

        w_sb = {}
        for name, wd in (("q", wq_d), ("k", wk_d), ("v", wv_d)):
            t = cp.tile([C, C], F32, tag=f"w_{name}")
            nc.sync.dma_start(t[:, :], wd[:, :])
            w_sb[name] = t

        # wT_aug: rows 0..63 = w^T (c, o), row 64 = bias (o)
        wT = {}
        with tc.tile_pool(name="pp0", bufs=2, space="PSUM") as pp0:
            for name, bd in (("q", bq_d), ("k", bk_d), ("v", bv_d)):
                st = cp.tile([C + 1, C], F32, tag=f"wTst_{name}")
                t = cp.tile([C + 1, C], F32R, tag=f"wT_{name}")
                ps = pp0.tile([C, C], F32, tag="wT_ps")
                nc.tensor.transpose(ps[:, :], w_sb[name][:, :], ident[0:C, 0:C])
                nc.vector.tensor_copy(st[0:C, :], ps[:, :])
                nc.sync.dma_start(st[C : C + 1, :], bd[:, :])
                nc.vector.tensor_copy(t[:, :], st[:, :])
                wT[name] = t

        # ---- projections (channel-major) and feature-major transposes ----
        Q_cm = cp.tile([C, HH * W], F32)
        K_cm = cp.tile([C, H * W], F32)
        V_cmb = cp.tile([C, H * W], BF16)
        # qT2: (w, i) duplicated on both partition halves (rhs of scores)
        # kT2: (w, j) even j-blocks on partitions 0-63, odd on 64-127 (lhsT)
        qT2 = cp.tile([128, NQ], F32R)
        kT2 = cp.tile([128, NK // 2], F32R)

        # vf_aug (128, JB, 65) bf16: partition p of block jb = v-token
        # (h = 2*jb + p//64, o = p%64); col 64 = 1.0 (denominator trick)
        vf = cp.tile([128, JB, 65], BF16)
        nc.vector.memset(vf[:, :, 64:65], 1.0)
        v_hview = V_cmb[:, :].rearrange("p (h2 h1 w) -> p h1 h2 w", h1=2, w=W)

        _cp_n = [0]

        def psum_copy(dst, src, allow_act=True):
            if _cp_n[0] % 2 == 0:
                nc.scalar.activation(dst, src, AF.Copy)
            else:
                nc.vector.tensor_copy(dst, src)
            _cp_n[0] += 1

        with tc.tile_pool(name="pp1", bufs=3, space="PSUM") as pp1:
            def project(dst, wTt, x_sb, ch, allow_act=False):
                ps = pp1.tile([C, 1024], F32, tag="setup")
                for c2 in range(2):
                    nc.tensor.matmul(
                        ps[:, ts(c2, 512)],
                        lhsT=wTt[:, :],
                        rhs=x_sb[:, ch * 1024 + c2 * 512 :][:, 0:512],
                        start=True, stop=True,
                    )
                psum_copy(dst[:, ts(ch, 1024)], ps[:, :], allow_act)

            # q transposes to psum base 0, then duplicate onto both
            # partition halves of qT2 (engine copies may cross bases)
            def q_transpose(grp, allow_act=False):
                ps = pp1.tile([64, 1024], F32, tag="setup")
                for hh in range(16):
                    h = grp * 16 + hh
                    nc.tensor.transpose(
                        ps[:, ts(hh, 64)], Q_cm[:, ts(h, 64)], ident[0:C, 0:C]
                    )
                psum_copy(qT2[0:64, ts(grp, 1024)], ps[:, :], allow_act)
                psum_copy(qT2[64:128, ts(grp, 1024)], ps[:, :], allow_act)

            # k transposes: h order in psum; block jb = h//2: even jb ->
            # kT2 partitions 0-63, odd jb -> 64-127 (strided split copies)
            def k_transpose(grp, allow_act=False):
                ps = pp1.tile([64, 1024], F32, tag="setup")
                for hh in range(16):
                    h = grp * 16 + hh
                    nc.tensor.transpose(
                        ps[:, ts(hh, 64)], K_cm[:, ts(h, 64)], ident[0:C, 0:C]
                    )
                pv = ps[:, :].rearrange("p (b two c) -> p b two c", two=2, c=128)
                for half in range(2):
                    dst = kT2[64 * half : 64 * half + 64, ts(grp, 512)].rearrange(
                        "p (b c) -> p b c", c=128
                    )
                    psum_copy(dst, pv[:, :, half, :], allow_act)

            # q path first (every scores matmul needs qT2)
            for ch in range(HH * W // 1024):
                project(Q_cm, wT["q"], x1_sb, ch, allow_act=True)
            for grp in range(HH // 16):
                q_transpose(grp, allow_act=True)
            # k and v interleaved per chunk; vf DMAs per 8-block group
            for ch in range(H * W // 1024):
                project(K_cm, wT["k"], x2_sb, ch, allow_act=(ch == 0))
                k_transpose(ch, allow_act=(ch == 0))
                project(V_cmb, wT["v"], x2_sb, ch)
                for h1 in range(2):
                    dst = vf[64 * h1 : 64 * (h1 + 1), ts(ch, 8), 0:W]
                    srcv = v_hview[:, h1, ts(ch, 8), :]
                    if h1 == 0:
                        nc.scalar.activation(dst, srcv, AF.Copy)
                    else:
                        nc.vector.tensor_copy(dst, srcv)

        # ---- main attention loop: 4 passes over i, row-packed j pairs ----
        # One PSUM tile per pair holds block A (cols 0-511) and block B
        # (cols 512-1023) at the same i-window: the two scores matmuls are
        # adjacent and overlap in the PE array (row groups 0-1 vs 2-3), and
        # a single FD=1024 exp covers both blocks.
        outT_sb = cp.tile([C + 1, NQ], F32)
        with (
            tc.tile_pool(name="outp", bufs=1, space="PSUM") as op_pool,
            tc.tile_pool(name="sp", bufs=3, space="PSUM") as sp,
            tc.tile_pool(name="ppool", bufs=4) as p_pool,
            tc.tile_pool(name="tp2", bufs=1, space="PSUM") as tp2,
            tc.tile_pool(name="opool", bufs=4) as o_pool,
            tc.tile_pool(name="rpool", bufs=4) as r_pool,
        ):
            for ih in range(NQ // IP):
                i0 = ih * IP
                outT_ps = op_pool.tile([C + 1, IP], F32, tag="outT")
                for p in range(NP):
                    sps = sp.tile([128, 2 * IP], F32, tag="scores")
                    for blk in range(2):
                        half = 64 * blk
                        nc.tensor.matmul(
                            sps[:, ts(blk, IP)],
                            lhsT=kT2[half : half + 64, ts(p, 128)],
                            rhs=qT2[half : half + 64, i0 : i0 + IP],
                            start=True, stop=True,
                        )
                    pt = p_pool.tile([128, 2 * IP], BF16, tag="p")
                    nc.scalar.activation(pt[:, :], sps[:, :], AF.Exp)
                    for blk in range(2):
                        jb = 2 * p + blk
                        nc.tensor.matmul(
                            outT_ps[:, :],
                            lhsT=vf[:, jb, :],
                            rhs=pt[:, ts(blk, IP)],
                            start=(p == 0 and blk == 0),
                            stop=(p == NP - 1 and blk == 1),
                        )
                # drain this pass's accumulator to SBUF, then normalize +
                # store its four output tiles while the next pass runs
                dst = outT_sb[:, i0 : i0 + IP]
                if ih % 2 == 0:
                    nc.scalar.activation(dst, outT_ps[:, :], AF.Copy)
                else:
                    nc.vector.tensor_copy(dst, outT_ps[:, :])
                for tt in range(IP // 128):
                    t = ih * (IP // 128) + tt
                    ps = tp2.tile([128, C + 1], F32, tag="ot")
                    nc.tensor.transpose(
                        ps[:, :], outT_sb[:, ts(t, 128)], ident[0 : C + 1, 0 : C + 1]
                    )
                    rec = r_pool.tile([128, 1], F32, tag="rec")
                    nc.vector.reciprocal(rec[:, :], ps[:, C : C + 1])
                    ot = o_pool.tile([128, C], F32, tag="o")
                    nc.vector.tensor_scalar_mul(ot[:, :], ps[:, 0:C], rec[:, 0:1])
                    # rows p = h_loc*64 + o  ->  out[o, 2t + h_loc, :]
                    dest = out_d[:, 2 * t : 2 * t + 2, :].rearrange("o h w -> h o w")
                    nc.sync.dma_start(dest, ot[:, :])

    nc.compile()
    return nc


def _get_nc():
    if "nc" not in _CACHE:
        _CACHE["nc"] = _build_nc()
    return _CACHE["nc"]


def _in_maps(v1, v2, wq, bq, wk, bk, wv, bv):
    maps = []
    for core in range(NCORES):
        b, half = divmod(core, 2)
        maps.append({
            "x1": np.ascontiguousarray(
                v1[b, :, half * HH : (half + 1) * HH, :], dtype=np.float32
            ).reshape(C, HH * W),
            "x2": np.ascontiguousarray(v2[b], dtype=np.float32).reshape(C, H * W),
            "wq": np.ascontiguousarray(wq, dtype=np.float32),
            "wk": np.ascontiguousarray(wk, dtype=np.float32),
            "wv": np.ascontiguousarray(wv, dtype=np.float32),
            "bq": np.ascontiguousarray(bq, dtype=np.float32).reshape(1, C),
            "bk": np.ascontiguousarray(bk, dtype=np.float32).reshape(1, C),
            "bv": np.ascontiguousarray(bv, dtype=np.float32).reshape(1, C),
        })
    return maps


def _gather(results, v1):
    out = np.zeros((B, C, H, W), dtype=np.float32)
    for core in range(NCORES):
        b, half = divmod(core, 2)
        out[b, :, half * HH : (half + 1) * HH, :] = results[core]["out"]
    return out


def _run(trace=False, **inputs):
    from concourse.bass_utils import run_bass_kernel_spmd

    nc = _get_nc()
    maps = _in_maps(**inputs)
    res = run_bass_kernel_spmd(
        nc, maps, core_ids=list(range(NCORES)), trace=trace
    )
    return _gather(res.results, inputs["v1"]), res


def kernel(**inputs):
    out, _ = _run(trace=False, **inputs)
    return out
